# revision 15
# baseline (speedup 1.0000x reference)
"""Mixture-of-Experts (B=4, S=2048, D=1024, F=4096, E=8, top-2) on 8 trn2 NeuronCores.

Strategy: expert parallelism with a capacity quota + F-sharded overflow.
  - Host: gate (softmax + top-2 + renorm) in float64, dispatch tokens per
    expert. Capacity Q is chosen to minimize Q + overflow cost; each core
    processes its own expert's first Q tokens (zero-padded below Q) with the
    full weights resident in SBUF as bf16.
  - Tokens beyond Q (overflow of hot experts) are F-sharded: every core
    holds a 512-wide F-slice of each overflowing expert's weights (2.1 MB)
    and computes a partial y for ALL overflow tokens; the host sums the 8
    partials. This keeps per-core work at Q + OF/8 token-equivalents while
    adding only ~16 KB/partition of SBUF and ~2 MB of DMA.
  - DMA queues: W1 + phase-2 weights + y-out on the scalar-engine queue;
    x + W2 on the sync-engine queue. Both start at t=0 so the first matmul
    issues at ~5 us and W2 lands long before mm2 of chunk 0 needs it.
  - mm1 of chunk c+1 runs 4 PSUM-groups ahead (gelu deferred) before mm2 of
    chunk c, so the gelu tail never bubbles the tensor engine.
"""

import copy
import sys

import numpy as np

for _p in ("/opt/trn_rl_repo", "/opt/pypackages"):
    if _p not in sys.path:
        sys.path.append(_p)

import ml_dtypes

B, S, D = 4, 2048, 1024
F = 4 * D
E = 8
TOP_K = 2
P = 128
KO = D // P        # 8  k-subtiles for mm1 (contraction D)
FT = F // P        # 32 f-tiles of h (own phase)
DT = D // P        # 8  d-tiles of y
FB = F // 512      # 8  f-blocks (w1 tile granularity)
FLOC = F // E      # 512 per-core F-slice for overflow phase
OFT = FLOC // P    # 4  f-tiles of overflow h
CC = 512           # chunk capacity (one PSUM bank of fp32)
LOOK = 3           # mm1 groups of chunk c+1 emitted before mm2 of chunk c
WARM = 22          # prologue matmuls on the first (x, w1) pieces: fill the
                   # weight-stream drip window and keep the HAM clock warm
LOOK1 = 12         # deeper lookahead into chunk 1: delays mm2(chunk 0) until
                   # the W2 stream (queued behind W1 + x1) has landed

# test-harness hooks (left off for grading)
TRACE = False
LAST_RESULTS = None

_compiled = {}


def _split_drain_waits(nc, max_waits=1):
    """This walrus build rejects instructions carrying more than one sync
    wait ("Too many sync wait commands"). Keep one wait on the instruction and
    move the excess onto NoOps inserted right before it on the same engine
    (engines are in-order, so blocking semantics are identical). Updates stay
    on the original instruction — moving them to a trailing NoOp could signal
    before the op's writes land."""
    import concourse.mybir as mybir

    m = nc.m
    new_module = copy.replace(m, functions=[])
    for function in m.functions:
        new_function = copy.replace(function, blocks=[])
        new_function.set_allocations_from_list(function.allocations)
        for block in function.blocks:
            out = []
            for inst in block.instructions:
                si = getattr(inst, "sync_info", None)
                on_wait = list(si.on_wait) if si is not None and si.on_wait else []
                if len(on_wait) > max_waits:
                    engine = getattr(inst, "engine", None)
                    extra, keep = on_wait[max_waits:], on_wait[:max_waits]
                    for j, w in enumerate(extra):
                        out.append(
                            mybir.InstNoOp(
                                name=f"{inst.name}-w{j}",
                                engine=engine,
                                sync_info=mybir.SyncInfo(on_wait=[w], on_update=[]),
                                bass_nofuse=True,
                            )
                        )
                    inst.sync_info = mybir.SyncInfo(
                        on_wait=keep,
                        on_update=list(si.on_update) if si.on_update else [],
                    )
                out.append(inst)
            new_function.blocks.append(copy.replace(block, instructions=out))
        new_module.functions.append(new_function)
    nc.m = new_module
    return nc


def _pick_quota(cnts):
    """Minimize per-core cost in column-equivalents: Q own columns, plus
    overflow columns at 1/8 weight, plus ~2.5us fixed per overflow segment."""
    best = None
    cands = sorted(set(list(cnts) + [max(cnts)]))
    for q in cands:
        if q < 640:
            continue
        of = sum(max(c - q, 0) for c in cnts)
        nseg = sum(1 for c in cnts if c > q)
        cost = q + 0.125 * of + 12.0 * nseg
        if best is None or cost < best[0]:
            best = (cost, q)
    return best[1]


def _chunk_sizes(q):
    """[512 x k, r, 128] summing to q, r in [129, 512] when possible."""
    if q <= CC:
        return [q]
    rem = q - 128
    sizes = []
    while rem > CC:
        sizes.append(CC)
        rem -= CC
    sizes.append(rem)
    sizes.append(128)
    if len(sizes) >= 4:
        # park the short chunk mid-stream where its bubbles are absorbed
        sizes = sizes[:2] + [sizes[-1]] + sizes[2:-1]
    return sizes


def _build_nc(q, cnts):
    import concourse.bass as bass
    import concourse.mybir as mybir
    from concourse.tile import TileContext

    fp32 = mybir.dt.float32
    bf16 = mybir.dt.bfloat16
    AF = mybir.ActivationFunctionType

    sizes = _chunk_sizes(q)
    NCH = len(sizes)
    ofs = [(e, max(cnts[e] - q, 0)) for e in range(E) if cnts[e] > q]
    KOV = len(ofs)
    # overflow chunks: (segment index, col offset within yo, cn)
    of_chunks = []
    yo_off = 0
    for si, (e, cnt) in enumerate(ofs):
        rem, off = cnt, 0
        n = -(-rem // CC)
        base, extra = divmod(rem, n)
        for i in range(n):
            cn = base + (1 if i < extra else 0)
            of_chunks.append((si, yo_off, cn))
            yo_off += cn
    oftot = yo_off

    nc = bass.Bass()
    x = nc.declare_dram_parameter("x", [P, NCH * KO * CC], bf16, isOutput=False)
    w1 = nc.declare_dram_parameter("w1", [P, KO * F], bf16, isOutput=False)
    w2 = nc.declare_dram_parameter("w2", [P, FT * D], bf16, isOutput=False)
    b1 = nc.declare_dram_parameter("b1", [P, FT], fp32, isOutput=False)
    b2 = nc.declare_dram_parameter("b2", [P, DT], fp32, isOutput=False)
    if KOV:
        xo = nc.declare_dram_parameter(
            "xo", [P, len(of_chunks) * KO * CC], bf16, isOutput=False)
        w1o = nc.declare_dram_parameter("w1o", [P, KOV * KO * FLOC], bf16,
                                        isOutput=False)
        w2o = nc.declare_dram_parameter("w2o", [P, KOV * OFT * D], bf16,
                                        isOutput=False)
        b1o = nc.declare_dram_parameter("b1o", [P, KOV * OFT], fp32,
                                        isOutput=False)
        b2o = nc.declare_dram_parameter("b2o", [P, KOV * DT], fp32,
                                        isOutput=False)
    y = nc.declare_dram_parameter("y", [D, q], bf16, isOutput=True)
    if KOV:
        yo = nc.declare_dram_parameter("yo", [D, oftot], bf16, isOutput=True)

    xr = x.rearrange("p (n ko c) -> p n ko c", ko=KO, c=CC)
    w1r = w1.rearrange("p (ko f) -> p ko f", ko=KO)
    w2r = w2.rearrange("p (fo d) -> p fo d", fo=FT)
    yr = y.rearrange("(dt di) c -> di dt c", di=P)
    if KOV:
        xor_ = xo.rearrange("p (n ko c) -> p n ko c", ko=KO, c=CC)
        w1or = w1o.rearrange("p (s ko f) -> p s ko f", s=KOV, ko=KO)
        w2or = w2o.rearrange("p (s fo d) -> p s fo d", s=KOV, fo=OFT)
        yor = yo.rearrange("(dt di) c -> di dt c", di=P)

    with TileContext(nc) as tc:
        with (
            tc.tile_pool(name="wpool", bufs=1) as wpool,
            tc.tile_pool(name="xpool", bufs=2) as xpool,
            tc.tile_pool(name="hpool", bufs=1) as hpool,
            tc.tile_pool(name="ypool", bufs=2) as ypool,
            tc.tile_pool(name="hpsum", bufs=5, space="PSUM") as hpsum,
            tc.tile_pool(name="ypsum", bufs=3, space="PSUM") as ypsum,
        ):
            # Sync queue carries the latency-critical stream in strict
            # priority order: x0, W1 waves, x1, W2, x2.., xo. The queues share
            # ~220 GB/s of HBM read bandwidth, so nothing else may compete
            # early. Scalar queue: biases now, overflow weights later, y-out.
            x_t = [None] * NCH
            x_t[0] = xpool.tile([P, KO, CC], bf16, tag="x", name="x_sb")
            w1_t = {}
            # interleave x0 and wave-0 per ko so the first mm1 group starts
            # as soon as its own (x piece, weight tile) pair has landed
            for ko in range(KO):
                nc.sync.dma_start(x_t[0][:, ko, :sizes[0]],
                                  xr[:, 0, ko, :sizes[0]])
                t = wpool.tile([P, 512], bf16, tag=f"w1_{ko}_0", name="w1_sb")
                nc.sync.dma_start(t[:], w1r[:, ko, 0:512])
                w1_t[(ko, 0)] = t
            for fb in range(1, FB):
                for ko in range(KO):
                    t = wpool.tile([P, 512], bf16, tag=f"w1_{ko}_{fb}")
                    nc.sync.dma_start(t[:], w1r[:, ko, fb * 512:(fb + 1) * 512])
                    w1_t[(ko, fb)] = t

            b1_sb = wpool.tile([P, FT], fp32)
            nc.scalar.dma_start(b1_sb[:], b1[:])
            b2_sb = wpool.tile([P, DT], fp32)
            nc.scalar.dma_start(b2_sb[:], b2[:])
            if KOV:
                b1o_sb = wpool.tile([P, KOV * OFT], fp32)
                nc.scalar.dma_start(b1o_sb[:], b1o[:])
                b2o_sb = wpool.tile([P, KOV * DT], fp32)
                nc.scalar.dma_start(b2o_sb[:], b2o[:])

            w2_t = [None] * FT
            w1o_sb = w2o_sb = None

            def mm1_group(cn, x_sb, ft):
                h_ps = hpsum.tile([P, CC], fp32, tag="hps")
                fb, fc = divmod(ft * P, 512)
                for ko in range(KO):
                    nc.tensor.matmul(
                        h_ps[:, :cn],
                        w1_t[(ko, fb)][:, fc:fc + P],
                        x_sb[:, ko, :cn],
                        start=(ko == 0),
                        stop=(ko == KO - 1),
                    )
                return h_ps

            def gelu(cn, h_ps, h_sb, ft, ft_sb):
                nc.scalar.activation(
                    h_sb[:, ft_sb, :cn], h_ps[:, :cn], AF.Gelu,
                    bias=b1_sb[:, ft:ft + 1],
                )

            def mm2(ci, cn, c0, srcs):
                # srcs: fo -> (sbuf tile, index in tile)
                for dt_ in range(DT):
                    y_ps = ypsum.tile([P, CC], fp32, tag="yps")
                    for fo in range(FT):
                        t, fi = srcs[fo]
                        nc.tensor.matmul(
                            y_ps[:, :cn],
                            w2_t[fo][:, dt_ * P:(dt_ + 1) * P],
                            t[:, fi, :cn],
                            start=(fo == 0),
                            stop=(fo == FT - 1),
                        )
                    y_sb = ypool.tile([P, CC], bf16, tag="y")
                    nc.vector.tensor_scalar_add(
                        y_sb[:, :cn], y_ps[:, :cn], b2_sb[:, dt_:dt_ + 1]
                    )
                    nc.scalar.dma_start(yr[:, dt_, c0:c0 + cn], y_sb[:, :cn])

            # ---- warmup: matmuls on the first landed (x, w1) pieces while
            # the rest of wave 0 streams in; results are never read
            for _ in range(WARM):
                j_ps = hpsum.tile([P, CC], fp32, tag="hps")
                nc.tensor.matmul(
                    j_ps[:, :sizes[0]], w1_t[(0, 0)][:, 0:P],
                    x_t[0][:, 0, :sizes[0]], start=True, stop=True,
                )

            # ---- own-expert phase, software-pipelined
            offs = [sum(sizes[:i]) for i in range(NCH)]
            h2_sb = hpool.tile([P, LOOK1, CC], bf16, tag="h2")
            h_prev = None   # (ci, cn, c0, srcs)
            for ci, cn in enumerate(sizes):
                if ci > 0:
                    x_t[ci] = xpool.tile([P, KO, CC], bf16, tag="x", name="x_sb")
                    nc.sync.dma_start(x_t[ci][:, :, :cn], xr[:, ci, :, :cn])
                if ci == 1:
                    # W2 lands behind W1 + x1; needed from mm2(chunk 0) on
                    for fo in range(FT):
                        t = wpool.tile([P, D], bf16, tag=f"w2_{fo}", name="w2_sb")
                        nc.sync.dma_start(t[:], w2r[:, fo, :])
                        w2_t[fo] = t
                if ci == min(2, NCH - 1) and KOV:
                    w1o_sb = wpool.tile([P, KOV, KO, FLOC], bf16, tag="w1o")
                    nc.scalar.dma_start(w1o_sb[:], w1or[:])
                    w2o_sb = wpool.tile([P, KOV, OFT, D], bf16, tag="w2o")
                    nc.scalar.dma_start(w2o_sb[:], w2or[:])
                h_sb = hpool.tile([P, FT, CC], bf16, tag="h")
                srcs = {}
                if h_prev is None:
                    look = 0
                elif ci == 1:
                    look = min(LOOK1, FT)
                else:
                    look = min(LOOK, FT)
                if ci == 1:
                    # deep lookahead with immediate gelu into the dedicated h2
                    for ft in range(look):
                        h_ps = mm1_group(cn, x_t[ci], ft)
                        gelu(cn, h_ps, h2_sb, ft, ft)
                        srcs[ft] = (h2_sb, ft)
                    mm2(*h_prev)
                else:
                    held = []
                    for ft in range(look):
                        held.append(mm1_group(cn, x_t[ci], ft))
                    if h_prev is not None:
                        mm2(*h_prev)
                    for ft in range(look):
                        gelu(cn, held[ft], h_sb, ft, ft)
                        srcs[ft] = (h_sb, ft)
                for ft in range(look, FT):
                    h_ps = mm1_group(cn, x_t[ci], ft)
                    gelu(cn, h_ps, h_sb, ft, ft)
                    srcs[ft] = (h_sb, ft)
                h_prev = (ci, cn, offs[ci], srcs)

            def mm2o(si, o0, cn, oh_sb):
                for dt_ in range(DT):
                    y_ps = ypsum.tile([P, CC], fp32, tag="yps")
                    for fo in range(OFT):
                        nc.tensor.matmul(
                            y_ps[:, :cn],
                            w2o_sb[:, si, fo, dt_ * P:(dt_ + 1) * P],
                            oh_sb[:, fo, :cn],
                            start=(fo == 0),
                            stop=(fo == OFT - 1),
                        )
                    y_sb = ypool.tile([P, CC], bf16, tag="y")
                    nc.vector.tensor_scalar_add(
                        y_sb[:, :cn], y_ps[:, :cn],
                        b2o_sb[:, si * DT + dt_:si * DT + dt_ + 1],
                    )
                    nc.scalar.dma_start(yor[:, dt_, o0:o0 + cn], y_sb[:, :cn])

            # ---- overflow phase (F-sharded partials), pipelined: each
            # of-chunk's mm1 is the lookahead covering the previous mm2
            of_prev = None
            for oc, (si, o0, cn) in enumerate(of_chunks):
                xo_sb = xpool.tile([P, KO, CC], bf16, tag="x")
                nc.sync.dma_start(xo_sb[:, :, :cn], xor_[:, oc, :, :cn])
                oh_sb = hpool.tile([P, LOOK1, CC], bf16, tag="h2")
                for ft in range(OFT):
                    h_ps = hpsum.tile([P, CC], fp32, tag="hps")
                    for ko in range(KO):
                        nc.tensor.matmul(
                            h_ps[:, :cn],
                            w1o_sb[:, si, ko, ft * P:(ft + 1) * P],
                            xo_sb[:, ko, :cn],
                            start=(ko == 0),
                            stop=(ko == KO - 1),
                        )
                    nc.scalar.activation(
                        oh_sb[:, ft, :cn], h_ps[:, :cn], AF.Gelu,
                        bias=b1o_sb[:, si * OFT + ft:si * OFT + ft + 1],
                    )
                if oc == 0:
                    mm2(*h_prev)
                    h_prev = None
                mm2o(si, o0, cn, oh_sb)
            if h_prev is not None:
                mm2(*h_prev)

    return _split_drain_waits(nc)


def _to_bf16(a):
    """Fast float32 -> bfloat16 with round-to-nearest-even via bit ops."""
    a = np.ascontiguousarray(a, dtype=np.float32)
    u = a.view(np.uint32)
    r = ((u + 0x7FFF + ((u >> 16) & 1)) >> 16).astype(np.uint16)
    return r.view(ml_dtypes.bfloat16)


def _pack_cols(mat_bf16, sizes):
    """mat [CT, D] bf16 -> [P, nch, KO, CC] padded chunk regions."""
    out = np.zeros((P, len(sizes), KO, CC), dtype=ml_dtypes.bfloat16)
    c0 = 0
    for i, cn in enumerate(sizes):
        blk = mat_bf16[c0:c0 + cn].T                  # [D, cn]
        out[:, i, :, :cn] = blk.reshape(KO, P, cn).transpose(1, 0, 2)
        c0 += cn
    return np.ascontiguousarray(out.reshape(P, -1))


def kernel(hidden_states, Wg, bg, W1, b1, W2, b2):
    from concourse import bass_utils

    hs = np.ascontiguousarray(hidden_states, dtype=np.float32).reshape(B * S, D)

    # ---- Gate on host (float64): softmax over experts, top-2, renormalize
    logits = hs.astype(np.float64) @ np.asarray(Wg, np.float64).T
    logits += np.asarray(bg, np.float64)
    logits -= logits.max(axis=-1, keepdims=True)
    p = np.exp(logits)
    p /= p.sum(axis=-1, keepdims=True)

    i1 = p.argmax(axis=-1)
    rows = np.arange(B * S)
    p1 = p[rows, i1]
    pm = p.copy()
    pm[rows, i1] = -1.0
    i2 = pm.argmax(axis=-1)
    p2 = p[rows, i2]
    denom = p1 + p2
    g1 = (p1 / denom).astype(np.float32)
    g2 = (p2 / denom).astype(np.float32)

    # ---- Dispatch: token ids + combine weights per expert
    ids, cws = [], []
    for e in range(E):
        m1 = np.nonzero(i1 == e)[0]
        m2 = np.nonzero(i2 == e)[0]
        ids.append(np.concatenate([m1, m2]))
        cws.append(np.concatenate([g1[m1], g2[m2]]))
    cnts = tuple(len(x) for x in ids)

    q = _pick_quota(cnts)
    sizes = _chunk_sizes(q)
    ofs = [(e, cnts[e] - q) for e in range(E) if cnts[e] > q]
    KOV = len(ofs)

    key = (q, cnts)
    if key not in _compiled:
        _compiled[key] = _build_nc(q, cnts)
    nc = _compiled[key]

    W1 = np.asarray(W1, np.float32)
    W2 = np.asarray(W2, np.float32)
    b1 = np.asarray(b1, np.float32)
    b2 = np.asarray(b2, np.float32)

    # overflow columns (shared across cores)
    if KOV:
        of_ids = np.concatenate([ids[e][q:] for e, _ in ofs])
        xo_all = _to_bf16(hs[of_ids])
        of_sizes = []
        for e, cnt in ofs:
            n = -(-cnt // CC)
            base, extra = divmod(cnt, n)
            of_sizes += [base + (1 if i < extra else 0) for i in range(n)]
        xo_host = _pack_cols(xo_all, of_sizes)

    in_maps = []
    for c in range(E):
        cnt = min(cnts[c], q)
        xc = np.zeros((q, D), dtype=ml_dtypes.bfloat16)
        xc[:cnt] = _to_bf16(hs[ids[c][:cnt]])
        m = {
            "x": _pack_cols(xc, sizes),
            "w1": np.ascontiguousarray(
                _to_bf16(W1[c]).reshape(KO, P, F).transpose(1, 0, 2).reshape(P, -1)),
            "w2": np.ascontiguousarray(
                _to_bf16(W2[c]).reshape(FT, P, D).transpose(1, 0, 2).reshape(P, -1)),
            "b1": np.ascontiguousarray(b1[c].reshape(FT, P).T),
            "b2": np.ascontiguousarray(b2[c].reshape(DT, P).T),
        }
        if KOV:
            sl = slice(c * FLOC, (c + 1) * FLOC)
            w1oh = np.empty((P, KOV, KO, FLOC), dtype=ml_dtypes.bfloat16)
            w2oh = np.empty((P, KOV, OFT, D), dtype=ml_dtypes.bfloat16)
            b1oh = np.empty((P, KOV, OFT), dtype=np.float32)
            b2oh = np.empty((P, KOV, DT), dtype=np.float32)
            for si, (e, _) in enumerate(ofs):
                w1oh[:, si] = _to_bf16(W1[e][:, sl]).reshape(
                    KO, P, FLOC).transpose(1, 0, 2)
                w2oh[:, si] = _to_bf16(W2[e][sl, :]).reshape(
                    OFT, P, D).transpose(1, 0, 2)
                b1oh[:, si] = b1[e, sl].reshape(OFT, P).T
                b2oh[:, si] = (b2[e] / E).reshape(DT, P).T
            m.update({
                "xo": xo_host,
                "w1o": np.ascontiguousarray(w1oh.reshape(P, -1)),
                "w2o": np.ascontiguousarray(w2oh.reshape(P, -1)),
                "b1o": np.ascontiguousarray(b1oh.reshape(P, -1)),
                "b2o": np.ascontiguousarray(b2oh.reshape(P, -1)),
            })
        in_maps.append(m)

    kwargs = {}
    if TRACE:
        import os as _os
        kwargs = dict(trace=True, trace_cores=list(range(E)))
        if _os.environ.get("MOE_TRACE_DIR"):
            _os.makedirs(_os.environ["MOE_TRACE_DIR"], exist_ok=True)
            kwargs["tmpdir"] = _os.environ["MOE_TRACE_DIR"]
    res = bass_utils.run_bass_kernel_spmd(nc, in_maps, list(range(E)), **kwargs)
    global LAST_RESULTS
    LAST_RESULTS = res

    # ---- Combine: own-phase outputs, then overflow partial sums
    out = np.zeros((B * S, D), dtype=np.float32)
    for e in range(E):
        cnt = min(cnts[e], q)
        ye = res.results[e]["y"].astype(np.float32)
        out[ids[e][:cnt]] += cws[e][:cnt, None] * ye[:, :cnt].T
    if KOV:
        yo_sum = np.zeros((D, sum(c for _, c in ofs)), dtype=np.float32)
        for c in range(E):
            yo_sum += res.results[c]["yo"].astype(np.float32)
        o0 = 0
        for e, cnt in ofs:
            out[ids[e][q:]] += cws[e][q:, None] * yo_sum[:, o0:o0 + cnt].T
            o0 += cnt
    return out.reshape(B, S, D)


# revision 21
# speedup vs baseline: 1.0088x; 1.0088x over previous
"""Mixture-of-Experts (B=4, S=2048, D=1024, F=4096, E=8, top-2) on 8 trn2 NeuronCores.

Strategy: expert parallelism with a capacity quota + F-sharded overflow.
  - Host: gate (softmax + top-2 + renorm) in float64, dispatch tokens per
    expert. Capacity Q is chosen to minimize Q + overflow cost; each core
    processes its own expert's first Q tokens (zero-padded below Q) with the
    full weights resident in SBUF as bf16.
  - Tokens beyond Q (overflow of hot experts) are F-sharded: every core
    holds a 512-wide F-slice of each overflowing expert's weights (2.1 MB)
    and computes a partial y for ALL overflow tokens; the host sums the 8
    partials. This keeps per-core work at Q + OF/8 token-equivalents while
    adding only ~16 KB/partition of SBUF and ~2 MB of DMA.
  - DMA queues: W1 + phase-2 weights + y-out on the scalar-engine queue;
    x + W2 on the sync-engine queue. Both start at t=0 so the first matmul
    issues at ~5 us and W2 lands long before mm2 of chunk 0 needs it.
  - mm1 of chunk c+1 runs 4 PSUM-groups ahead (gelu deferred) before mm2 of
    chunk c, so the gelu tail never bubbles the tensor engine.
"""

import copy
import sys

import numpy as np

for _p in ("/opt/trn_rl_repo", "/opt/pypackages"):
    if _p not in sys.path:
        sys.path.append(_p)

import ml_dtypes

B, S, D = 4, 2048, 1024
F = 4 * D
E = 8
TOP_K = 2
P = 128
KO = D // P        # 8  k-subtiles for mm1 (contraction D)
FT = F // P        # 32 f-tiles of h (own phase)
DT = D // P        # 8  d-tiles of y
FB = F // 512      # 8  f-blocks (w1 tile granularity)
FLOC = F // E      # 512 per-core F-slice for overflow phase
OFT = FLOC // P    # 4  f-tiles of overflow h
CC = 512           # chunk capacity (one PSUM bank of fp32)
LOOK = 3           # mm1 groups of chunk c+1 emitted before mm2 of chunk c
BLK0 = 4           # chunk-0 leading f-tiles computed ko-outer so each
                   # arriving (x, w1) piece feeds matmuls immediately
LOOK1 = 12         # deeper lookahead into chunk 1: delays mm2(chunk 0) until
                   # the W2 stream (queued behind W1 + x1) has landed

# test-harness hooks (left off for grading)
TRACE = False
LAST_RESULTS = None

_compiled = {}


def _split_drain_waits(nc, max_waits=1):
    """This walrus build rejects instructions carrying more than one sync
    wait ("Too many sync wait commands"). Keep one wait on the instruction and
    move the excess onto NoOps inserted right before it on the same engine
    (engines are in-order, so blocking semantics are identical). Updates stay
    on the original instruction — moving them to a trailing NoOp could signal
    before the op's writes land."""
    import concourse.mybir as mybir

    m = nc.m
    new_module = copy.replace(m, functions=[])
    for function in m.functions:
        new_function = copy.replace(function, blocks=[])
        new_function.set_allocations_from_list(function.allocations)
        for block in function.blocks:
            out = []
            for inst in block.instructions:
                si = getattr(inst, "sync_info", None)
                on_wait = list(si.on_wait) if si is not None and si.on_wait else []
                if len(on_wait) > max_waits:
                    engine = getattr(inst, "engine", None)
                    extra, keep = on_wait[max_waits:], on_wait[:max_waits]
                    for j, w in enumerate(extra):
                        out.append(
                            mybir.InstNoOp(
                                name=f"{inst.name}-w{j}",
                                engine=engine,
                                sync_info=mybir.SyncInfo(on_wait=[w], on_update=[]),
                                bass_nofuse=True,
                            )
                        )
                    inst.sync_info = mybir.SyncInfo(
                        on_wait=keep,
                        on_update=list(si.on_update) if si.on_update else [],
                    )
                out.append(inst)
            new_function.blocks.append(copy.replace(block, instructions=out))
        new_module.functions.append(new_function)
    nc.m = new_module
    return nc


def _pick_quota(cnts):
    """Minimize per-core cost in column-equivalents: Q own columns, plus
    overflow columns at 1/8 weight, plus ~2.5us fixed per overflow segment."""
    best = None
    cands = sorted(set(list(cnts) + [max(cnts)]))
    for q in cands:
        if q < 640:
            continue
        of = sum(max(c - q, 0) for c in cnts)
        nseg = sum(1 for c in cnts if c > q)
        cost = q + 0.125 * of + 12.0 * nseg
        if best is None or cost < best[0]:
            best = (cost, q)
    return best[1]


def _chunk_sizes(q):
    """[512 x k, r, 128] summing to q, r in [129, 512] when possible."""
    if q <= CC:
        return [q]
    rem = q - 128
    sizes = []
    while rem > CC:
        sizes.append(CC)
        rem -= CC
    sizes.append(rem)
    sizes.append(128)
    return sizes


def _build_nc(q, cnts):
    import concourse.bass as bass
    import concourse.mybir as mybir
    from concourse.tile import TileContext

    fp32 = mybir.dt.float32
    bf16 = mybir.dt.bfloat16
    AF = mybir.ActivationFunctionType

    sizes = _chunk_sizes(q)
    NCH = len(sizes)
    ofs = [(e, max(cnts[e] - q, 0)) for e in range(E) if cnts[e] > q]
    KOV = len(ofs)
    # overflow chunks: (segment index, col offset within yo, cn)
    of_chunks = []
    yo_off = 0
    for si, (e, cnt) in enumerate(ofs):
        rem, off = cnt, 0
        n = -(-rem // CC)
        base, extra = divmod(rem, n)
        for i in range(n):
            cn = base + (1 if i < extra else 0)
            of_chunks.append((si, yo_off, cn))
            yo_off += cn
    oftot = yo_off

    nc = bass.Bass()
    x = nc.declare_dram_parameter("x", [P, NCH * KO * CC], bf16, isOutput=False)
    w1 = nc.declare_dram_parameter("w1", [P, KO * F], bf16, isOutput=False)
    w2 = nc.declare_dram_parameter("w2", [P, FT * D], bf16, isOutput=False)
    b1 = nc.declare_dram_parameter("b1", [P, FT], fp32, isOutput=False)
    b2 = nc.declare_dram_parameter("b2", [P, DT], fp32, isOutput=False)
    if KOV:
        xo = nc.declare_dram_parameter(
            "xo", [P, len(of_chunks) * KO * CC], bf16, isOutput=False)
        w1o = nc.declare_dram_parameter("w1o", [P, KOV * KO * FLOC], bf16,
                                        isOutput=False)
        w2o = nc.declare_dram_parameter("w2o", [P, KOV * OFT * D], bf16,
                                        isOutput=False)
        b1o = nc.declare_dram_parameter("b1o", [P, KOV * OFT], fp32,
                                        isOutput=False)
        b2o = nc.declare_dram_parameter("b2o", [P, KOV * DT], fp32,
                                        isOutput=False)
    y = nc.declare_dram_parameter("y", [D, q], bf16, isOutput=True)
    if KOV:
        yo = nc.declare_dram_parameter("yo", [D, oftot], bf16, isOutput=True)

    xr = x.rearrange("p (n ko c) -> p n ko c", ko=KO, c=CC)
    w1r = w1.rearrange("p (ko f) -> p ko f", ko=KO)
    w2r = w2.rearrange("p (fo d) -> p fo d", fo=FT)
    yr = y.rearrange("(dt di) c -> di dt c", di=P)
    if KOV:
        xor_ = xo.rearrange("p (n ko c) -> p n ko c", ko=KO, c=CC)
        w1or = w1o.rearrange("p (s ko f) -> p s ko f", s=KOV, ko=KO)
        w2or = w2o.rearrange("p (s fo d) -> p s fo d", s=KOV, fo=OFT)
        yor = yo.rearrange("(dt di) c -> di dt c", di=P)

    with TileContext(nc) as tc:
        with (
            tc.tile_pool(name="wpool", bufs=1) as wpool,
            tc.tile_pool(name="xpool", bufs=2) as xpool,
            tc.tile_pool(name="hpool", bufs=1) as hpool,
            tc.tile_pool(name="ypool", bufs=3) as ypool,
            tc.tile_pool(name="hpsum", bufs=5, space="PSUM") as hpsum,
            tc.tile_pool(name="ypsum", bufs=3, space="PSUM") as ypsum,
        ):
            # Sync queue carries the latency-critical stream in strict
            # priority order: x0, W1 waves, x1, W2, x2.., xo. The queues share
            # ~220 GB/s of HBM read bandwidth, so nothing else may compete
            # early. Scalar queue: biases now, overflow weights later, y-out.
            x_t = [None] * NCH
            x_t[0] = xpool.tile([P, KO, CC], bf16, tag="x", name="x_sb")
            w1_t = {}
            # interleave x0 and wave-0 per ko so the first mm1 group starts
            # as soon as its own (x piece, weight tile) pair has landed
            for ko in range(KO):
                nc.sync.dma_start(x_t[0][:, ko, :sizes[0]],
                                  xr[:, 0, ko, :sizes[0]])
                t = wpool.tile([P, 512], bf16, tag=f"w1_{ko}_0", name="w1_sb")
                nc.sync.dma_start(t[:], w1r[:, ko, 0:512])
                w1_t[(ko, 0)] = t
            for fb in range(1, FB):
                for ko in range(KO):
                    t = wpool.tile([P, 512], bf16, tag=f"w1_{ko}_{fb}")
                    nc.sync.dma_start(t[:], w1r[:, ko, fb * 512:(fb + 1) * 512])
                    w1_t[(ko, fb)] = t

            b1_sb = wpool.tile([P, FT], fp32)
            nc.scalar.dma_start(b1_sb[:], b1[:])
            b2_sb = wpool.tile([P, DT], fp32)
            nc.scalar.dma_start(b2_sb[:], b2[:])
            if KOV:
                b1o_sb = wpool.tile([P, KOV * OFT], fp32)
                nc.scalar.dma_start(b1o_sb[:], b1o[:])
                b2o_sb = wpool.tile([P, KOV * DT], fp32)
                nc.scalar.dma_start(b2o_sb[:], b2o[:])

            w2_t = [None] * FT
            w1o_sb = w2o_sb = None

            def mm1_group(cn, x_sb, ft):
                h_ps = hpsum.tile([P, CC], fp32, tag="hps")
                fb, fc = divmod(ft * P, 512)
                for ko in range(KO):
                    nc.tensor.matmul(
                        h_ps[:, :cn],
                        w1_t[(ko, fb)][:, fc:fc + P],
                        x_sb[:, ko, :cn],
                        start=(ko == 0),
                        stop=(ko == KO - 1),
                    )
                return h_ps

            def gelu(cn, h_ps, h_sb, ft, ft_sb):
                nc.scalar.activation(
                    h_sb[:, ft_sb, :cn], h_ps[:, :cn], AF.Gelu,
                    bias=b1_sb[:, ft:ft + 1],
                )

            def mm2(ci, cn, c0, srcs):
                # srcs: fo -> (sbuf tile, index in tile)
                for dt_ in range(DT):
                    y_ps = ypsum.tile([P, CC], fp32, tag="yps")
                    for fo in range(FT):
                        t, fi = srcs[fo]
                        nc.tensor.matmul(
                            y_ps[:, :cn],
                            w2_t[fo][:, dt_ * P:(dt_ + 1) * P],
                            t[:, fi, :cn],
                            start=(fo == 0),
                            stop=(fo == FT - 1),
                        )
                    y_sb = ypool.tile([P, CC], bf16, tag="y")
                    nc.vector.tensor_scalar_add(
                        y_sb[:, :cn], y_ps[:, :cn], b2_sb[:, dt_:dt_ + 1]
                    )
                    nc.scalar.dma_start(yr[:, dt_, c0:c0 + cn], y_sb[:, :cn])

            # ---- own-expert phase, software-pipelined
            offs = [sum(sizes[:i]) for i in range(NCH)]
            h2_sb = hpool.tile([P, LOOK1, CC], bf16, tag="h2")
            h_prev = None   # (ci, cn, c0, srcs)
            for ci, cn in enumerate(sizes):
                if ci > 0:
                    x_t[ci] = xpool.tile([P, KO, CC], bf16, tag="x", name="x_sb")
                    nc.sync.dma_start(x_t[ci][:, :, :cn], xr[:, ci, :, :cn])
                if ci == 1:
                    # W2 lands behind W1 + x1; needed from mm2(chunk 0) on
                    for fo in range(FT):
                        t = wpool.tile([P, D], bf16, tag=f"w2_{fo}", name="w2_sb")
                        nc.sync.dma_start(t[:], w2r[:, fo, :])
                        w2_t[fo] = t
                if ci == min(2, NCH - 1) and KOV:
                    w1o_sb = wpool.tile([P, KOV, KO, FLOC], bf16, tag="w1o")
                    nc.scalar.dma_start(w1o_sb[:], w1or[:])
                    w2o_sb = wpool.tile([P, KOV, OFT, D], bf16, tag="w2o")
                    nc.scalar.dma_start(w2o_sb[:], w2or[:])
                h_sb = hpool.tile([P, FT, CC], bf16, tag="h")
                srcs = {}
                if h_prev is None:
                    look = 0
                elif ci == 1:
                    look = min(LOOK1, FT)
                else:
                    look = min(LOOK, FT)
                if ci == 0:
                    # ko-outer over the first BLK0 f-tiles: consume each
                    # (x piece, w1 tile) pair of wave 0 the moment it lands
                    blk = []
                    for ft in range(BLK0):
                        h_ps = hpsum.tile([P, CC], fp32, tag="hps")
                        blk.append(h_ps)
                    for ko in range(KO):
                        for ft in range(BLK0):
                            nc.tensor.matmul(
                                blk[ft][:, :cn],
                                w1_t[(ko, 0)][:, ft * P:(ft + 1) * P],
                                x_t[0][:, ko, :cn],
                                start=(ko == 0),
                                stop=(ko == KO - 1),
                            )
                    for ft in range(BLK0):
                        gelu(cn, blk[ft], h_sb, ft, ft)
                        srcs[ft] = (h_sb, ft)
                    look = BLK0
                elif ci == 1:
                    # deep lookahead with immediate gelu into the dedicated h2
                    for ft in range(look):
                        h_ps = mm1_group(cn, x_t[ci], ft)
                        gelu(cn, h_ps, h2_sb, ft, ft)
                        srcs[ft] = (h2_sb, ft)
                    mm2(*h_prev)
                else:
                    held = []
                    for ft in range(look):
                        held.append(mm1_group(cn, x_t[ci], ft))
                    if h_prev is not None:
                        mm2(*h_prev)
                    for ft in range(look):
                        gelu(cn, held[ft], h_sb, ft, ft)
                        srcs[ft] = (h_sb, ft)
                for ft in range(look, FT):
                    h_ps = mm1_group(cn, x_t[ci], ft)
                    gelu(cn, h_ps, h_sb, ft, ft)
                    srcs[ft] = (h_sb, ft)
                h_prev = (ci, cn, offs[ci], srcs)

            def mm2o(si, o0, cn, oh_sb):
                for dt_ in range(DT):
                    y_ps = ypsum.tile([P, CC], fp32, tag="yps")
                    for fo in range(OFT):
                        nc.tensor.matmul(
                            y_ps[:, :cn],
                            w2o_sb[:, si, fo, dt_ * P:(dt_ + 1) * P],
                            oh_sb[:, fo, :cn],
                            start=(fo == 0),
                            stop=(fo == OFT - 1),
                        )
                    y_sb = ypool.tile([P, CC], bf16, tag="y")
                    nc.vector.tensor_scalar_add(
                        y_sb[:, :cn], y_ps[:, :cn],
                        b2o_sb[:, si * DT + dt_:si * DT + dt_ + 1],
                    )
                    nc.scalar.dma_start(yor[:, dt_, o0:o0 + cn], y_sb[:, :cn])

            # ---- overflow phase (F-sharded partials), pipelined: each
            # of-chunk's mm1 is the lookahead covering the previous mm2
            of_prev = None
            for oc, (si, o0, cn) in enumerate(of_chunks):
                xo_sb = xpool.tile([P, KO, CC], bf16, tag="x")
                nc.sync.dma_start(xo_sb[:, :, :cn], xor_[:, oc, :, :cn])
                oh_sb = hpool.tile([P, LOOK1, CC], bf16, tag="h2")
                for ft in range(OFT):
                    h_ps = hpsum.tile([P, CC], fp32, tag="hps")
                    for ko in range(KO):
                        nc.tensor.matmul(
                            h_ps[:, :cn],
                            w1o_sb[:, si, ko, ft * P:(ft + 1) * P],
                            xo_sb[:, ko, :cn],
                            start=(ko == 0),
                            stop=(ko == KO - 1),
                        )
                    nc.scalar.activation(
                        oh_sb[:, ft, :cn], h_ps[:, :cn], AF.Gelu,
                        bias=b1o_sb[:, si * OFT + ft:si * OFT + ft + 1],
                    )
                if oc == 0:
                    mm2(*h_prev)
                    h_prev = None
                mm2o(si, o0, cn, oh_sb)
            if h_prev is not None:
                mm2(*h_prev)

    return _split_drain_waits(nc)


def _to_bf16(a):
    """Fast float32 -> bfloat16 with round-to-nearest-even via bit ops."""
    a = np.ascontiguousarray(a, dtype=np.float32)
    u = a.view(np.uint32)
    r = ((u + 0x7FFF + ((u >> 16) & 1)) >> 16).astype(np.uint16)
    return r.view(ml_dtypes.bfloat16)


def _pack_cols(mat_bf16, sizes):
    """mat [CT, D] bf16 -> [P, nch, KO, CC] padded chunk regions."""
    out = np.zeros((P, len(sizes), KO, CC), dtype=ml_dtypes.bfloat16)
    c0 = 0
    for i, cn in enumerate(sizes):
        blk = mat_bf16[c0:c0 + cn].T                  # [D, cn]
        out[:, i, :, :cn] = blk.reshape(KO, P, cn).transpose(1, 0, 2)
        c0 += cn
    return np.ascontiguousarray(out.reshape(P, -1))


def kernel(hidden_states, Wg, bg, W1, b1, W2, b2):
    from concourse import bass_utils

    hs = np.ascontiguousarray(hidden_states, dtype=np.float32).reshape(B * S, D)

    # ---- Gate on host (float64): softmax over experts, top-2, renormalize
    logits = hs.astype(np.float64) @ np.asarray(Wg, np.float64).T
    logits += np.asarray(bg, np.float64)
    logits -= logits.max(axis=-1, keepdims=True)
    p = np.exp(logits)
    p /= p.sum(axis=-1, keepdims=True)

    i1 = p.argmax(axis=-1)
    rows = np.arange(B * S)
    p1 = p[rows, i1]
    pm = p.copy()
    pm[rows, i1] = -1.0
    i2 = pm.argmax(axis=-1)
    p2 = p[rows, i2]
    denom = p1 + p2
    g1 = (p1 / denom).astype(np.float32)
    g2 = (p2 / denom).astype(np.float32)

    # ---- Dispatch: token ids + combine weights per expert
    ids, cws = [], []
    for e in range(E):
        m1 = np.nonzero(i1 == e)[0]
        m2 = np.nonzero(i2 == e)[0]
        ids.append(np.concatenate([m1, m2]))
        cws.append(np.concatenate([g1[m1], g2[m2]]))
    cnts = tuple(len(x) for x in ids)

    q = _pick_quota(cnts)
    sizes = _chunk_sizes(q)
    ofs = [(e, cnts[e] - q) for e in range(E) if cnts[e] > q]
    KOV = len(ofs)

    key = (q, cnts)
    if key not in _compiled:
        _compiled[key] = _build_nc(q, cnts)
    nc = _compiled[key]

    W1 = np.asarray(W1, np.float32)
    W2 = np.asarray(W2, np.float32)
    b1 = np.asarray(b1, np.float32)
    b2 = np.asarray(b2, np.float32)

    # overflow columns (shared across cores)
    if KOV:
        of_ids = np.concatenate([ids[e][q:] for e, _ in ofs])
        xo_all = _to_bf16(hs[of_ids])
        of_sizes = []
        for e, cnt in ofs:
            n = -(-cnt // CC)
            base, extra = divmod(cnt, n)
            of_sizes += [base + (1 if i < extra else 0) for i in range(n)]
        xo_host = _pack_cols(xo_all, of_sizes)

    in_maps = []
    for c in range(E):
        cnt = min(cnts[c], q)
        xc = np.zeros((q, D), dtype=ml_dtypes.bfloat16)
        xc[:cnt] = _to_bf16(hs[ids[c][:cnt]])
        m = {
            "x": _pack_cols(xc, sizes),
            "w1": np.ascontiguousarray(
                _to_bf16(W1[c]).reshape(KO, P, F).transpose(1, 0, 2).reshape(P, -1)),
            "w2": np.ascontiguousarray(
                _to_bf16(W2[c]).reshape(FT, P, D).transpose(1, 0, 2).reshape(P, -1)),
            "b1": np.ascontiguousarray(b1[c].reshape(FT, P).T),
            "b2": np.ascontiguousarray(b2[c].reshape(DT, P).T),
        }
        if KOV:
            sl = slice(c * FLOC, (c + 1) * FLOC)
            w1oh = np.empty((P, KOV, KO, FLOC), dtype=ml_dtypes.bfloat16)
            w2oh = np.empty((P, KOV, OFT, D), dtype=ml_dtypes.bfloat16)
            b1oh = np.empty((P, KOV, OFT), dtype=np.float32)
            b2oh = np.empty((P, KOV, DT), dtype=np.float32)
            for si, (e, _) in enumerate(ofs):
                w1oh[:, si] = _to_bf16(W1[e][:, sl]).reshape(
                    KO, P, FLOC).transpose(1, 0, 2)
                w2oh[:, si] = _to_bf16(W2[e][sl, :]).reshape(
                    OFT, P, D).transpose(1, 0, 2)
                b1oh[:, si] = b1[e, sl].reshape(OFT, P).T
                b2oh[:, si] = (b2[e] / E).reshape(DT, P).T
            m.update({
                "xo": xo_host,
                "w1o": np.ascontiguousarray(w1oh.reshape(P, -1)),
                "w2o": np.ascontiguousarray(w2oh.reshape(P, -1)),
                "b1o": np.ascontiguousarray(b1oh.reshape(P, -1)),
                "b2o": np.ascontiguousarray(b2oh.reshape(P, -1)),
            })
        in_maps.append(m)

    kwargs = {}
    if TRACE:
        import os as _os
        kwargs = dict(trace=True, trace_cores=list(range(E)))
        if _os.environ.get("MOE_TRACE_DIR"):
            _os.makedirs(_os.environ["MOE_TRACE_DIR"], exist_ok=True)
            kwargs["tmpdir"] = _os.environ["MOE_TRACE_DIR"]
    res = bass_utils.run_bass_kernel_spmd(nc, in_maps, list(range(E)), **kwargs)
    global LAST_RESULTS
    LAST_RESULTS = res

    # ---- Combine: own-phase outputs, then overflow partial sums
    out = np.zeros((B * S, D), dtype=np.float32)
    for e in range(E):
        cnt = min(cnts[e], q)
        ye = res.results[e]["y"].astype(np.float32)
        out[ids[e][:cnt]] += cws[e][:cnt, None] * ye[:, :cnt].T
    if KOV:
        yo_sum = np.zeros((D, sum(c for _, c in ofs)), dtype=np.float32)
        for c in range(E):
            yo_sum += res.results[c]["yo"].astype(np.float32)
        o0 = 0
        for e, cnt in ofs:
            out[ids[e][q:]] += cws[e][q:, None] * yo_sum[:, o0:o0 + cnt].T
            o0 += cnt
    return out.reshape(B, S, D)


# revision 24
# speedup vs baseline: 1.0160x; 1.0072x over previous
"""Mixture-of-Experts (B=4, S=2048, D=1024, F=4096, E=8, top-2) on 8 trn2 NeuronCores.

Strategy: expert parallelism with a capacity quota + F-sharded overflow.
  - Host: gate (softmax + top-2 + renorm) in float64, dispatch tokens per
    expert. Capacity Q is chosen to minimize Q + overflow cost; each core
    processes its own expert's first Q tokens (zero-padded below Q) with the
    full weights resident in SBUF as bf16.
  - Tokens beyond Q (overflow of hot experts) are F-sharded: every core
    holds a 512-wide F-slice of each overflowing expert's weights (2.1 MB)
    and computes a partial y for ALL overflow tokens; the host sums the 8
    partials. This keeps per-core work at Q + OF/8 token-equivalents while
    adding only ~16 KB/partition of SBUF and ~2 MB of DMA.
  - DMA queues: W1 + phase-2 weights + y-out on the scalar-engine queue;
    x + W2 on the sync-engine queue. Both start at t=0 so the first matmul
    issues at ~5 us and W2 lands long before mm2 of chunk 0 needs it.
  - mm1 of chunk c+1 runs 4 PSUM-groups ahead (gelu deferred) before mm2 of
    chunk c, so the gelu tail never bubbles the tensor engine.
"""

import copy
import sys

import numpy as np

for _p in ("/opt/trn_rl_repo", "/opt/pypackages"):
    if _p not in sys.path:
        sys.path.append(_p)

import ml_dtypes

B, S, D = 4, 2048, 1024
F = 4 * D
E = 8
TOP_K = 2
P = 128
KO = D // P        # 8  k-subtiles for mm1 (contraction D)
FT = F // P        # 32 f-tiles of h (own phase)
DT = D // P        # 8  d-tiles of y
FB = F // 512      # 8  f-blocks (w1 tile granularity)
FLOC = F // E      # 512 per-core F-slice for overflow phase
OFT = FLOC // P    # 4  f-tiles of overflow h
CC = 512           # chunk capacity (one PSUM bank of fp32)
LOOK = 3           # mm1 groups of chunk c+1 emitted before mm2 of chunk c
BLK0 = 4           # chunk-0 leading f-tiles computed ko-outer so each
                   # arriving (x, w1) piece feeds matmuls immediately
LOOK1 = 12         # deeper lookahead into chunk 1: delays mm2(chunk 0) until
                   # the W2 stream (queued behind W1 + x1) has landed

# test-harness hooks (left off for grading)
TRACE = False
LAST_RESULTS = None

_compiled = {}


def _split_drain_waits(nc, max_waits=1):
    """This walrus build rejects instructions carrying more than one sync
    wait ("Too many sync wait commands"). Keep one wait on the instruction and
    move the excess onto NoOps inserted right before it on the same engine
    (engines are in-order, so blocking semantics are identical). Updates stay
    on the original instruction — moving them to a trailing NoOp could signal
    before the op's writes land."""
    import concourse.mybir as mybir

    m = nc.m
    new_module = copy.replace(m, functions=[])
    for function in m.functions:
        new_function = copy.replace(function, blocks=[])
        new_function.set_allocations_from_list(function.allocations)
        for block in function.blocks:
            out = []
            for inst in block.instructions:
                si = getattr(inst, "sync_info", None)
                on_wait = list(si.on_wait) if si is not None and si.on_wait else []
                if len(on_wait) > max_waits:
                    engine = getattr(inst, "engine", None)
                    extra, keep = on_wait[max_waits:], on_wait[:max_waits]
                    for j, w in enumerate(extra):
                        out.append(
                            mybir.InstNoOp(
                                name=f"{inst.name}-w{j}",
                                engine=engine,
                                sync_info=mybir.SyncInfo(on_wait=[w], on_update=[]),
                                bass_nofuse=True,
                            )
                        )
                    inst.sync_info = mybir.SyncInfo(
                        on_wait=keep,
                        on_update=list(si.on_update) if si.on_update else [],
                    )
                out.append(inst)
            new_function.blocks.append(copy.replace(block, instructions=out))
        new_module.functions.append(new_function)
    nc.m = new_module
    return nc


def _pick_quota(cnts):
    """Minimize per-core cost in column-equivalents: Q own columns, plus
    overflow columns at 1/8 weight, plus ~2.5us fixed per overflow segment."""
    best = None
    cands = sorted(set(list(cnts) + [max(cnts)]))
    for q in cands:
        if q < 640:
            continue
        of = sum(max(c - q, 0) for c in cnts)
        nseg = sum(1 for c in cnts if c > q)
        cost = q + 0.125 * of + 12.0 * nseg
        if best is None or cost < best[0]:
            best = (cost, q)
    return best[1]


def _chunk_sizes(q):
    """[512 x k, r, 128] summing to q, r in [129, 512] when possible."""
    if q <= CC:
        return [q]
    rem = q - 128
    sizes = []
    while rem > CC:
        sizes.append(CC)
        rem -= CC
    sizes.append(rem)
    sizes.append(128)
    return sizes


def _build_nc(q, cnts):
    import concourse.bass as bass
    import concourse.mybir as mybir
    from concourse.tile import TileContext

    fp32 = mybir.dt.float32
    bf16 = mybir.dt.bfloat16
    AF = mybir.ActivationFunctionType

    sizes = _chunk_sizes(q)
    NCH = len(sizes)
    ofs = [(e, max(cnts[e] - q, 0)) for e in range(E) if cnts[e] > q]
    KOV = len(ofs)
    # overflow chunks: (segment index, col offset within yo, cn)
    of_chunks = []
    yo_off = 0
    for si, (e, cnt) in enumerate(ofs):
        rem, off = cnt, 0
        n = -(-rem // CC)
        base, extra = divmod(rem, n)
        for i in range(n):
            cn = base + (1 if i < extra else 0)
            of_chunks.append((si, yo_off, cn))
            yo_off += cn
    oftot = yo_off

    nc = bass.Bass()
    x = nc.declare_dram_parameter("x", [P, NCH * KO * CC], bf16, isOutput=False)
    w1 = nc.declare_dram_parameter("w1", [P, KO * F], bf16, isOutput=False)
    w2 = nc.declare_dram_parameter("w2", [P, FT * D], bf16, isOutput=False)
    b1 = nc.declare_dram_parameter("b1", [P, FT], fp32, isOutput=False)
    b2 = nc.declare_dram_parameter("b2", [P, DT], fp32, isOutput=False)
    if KOV:
        xo = nc.declare_dram_parameter(
            "xo", [P, len(of_chunks) * KO * CC], bf16, isOutput=False)
        w1o = nc.declare_dram_parameter("w1o", [P, KOV * KO * FLOC], bf16,
                                        isOutput=False)
        w2o = nc.declare_dram_parameter("w2o", [P, KOV * OFT * D], bf16,
                                        isOutput=False)
        b1o = nc.declare_dram_parameter("b1o", [P, KOV * OFT], fp32,
                                        isOutput=False)
        b2o = nc.declare_dram_parameter("b2o", [P, KOV * DT], fp32,
                                        isOutput=False)
    y = nc.declare_dram_parameter("y", [D, q], bf16, isOutput=True)
    if KOV:
        yo = nc.declare_dram_parameter("yo", [D, oftot], bf16, isOutput=True)

    xr = x.rearrange("p (n ko c) -> p n ko c", ko=KO, c=CC)
    w1r = w1.rearrange("p (ko f) -> p ko f", ko=KO)
    w2r = w2.rearrange("p (fo d) -> p fo d", fo=FT)
    yr = y.rearrange("(dt di) c -> di dt c", di=P)
    if KOV:
        xor_ = xo.rearrange("p (n ko c) -> p n ko c", ko=KO, c=CC)
        w1or = w1o.rearrange("p (s ko f) -> p s ko f", s=KOV, ko=KO)
        w2or = w2o.rearrange("p (s fo d) -> p s fo d", s=KOV, fo=OFT)
        yor = yo.rearrange("(dt di) c -> di dt c", di=P)

    with TileContext(nc) as tc:
        with (
            tc.tile_pool(name="wpool", bufs=1) as wpool,
            tc.tile_pool(name="xpool", bufs=2) as xpool,
            tc.tile_pool(name="hpool", bufs=1) as hpool,
            tc.tile_pool(name="ypool", bufs=3) as ypool,
            tc.tile_pool(name="hpsum", bufs=6, space="PSUM") as hpsum,
            tc.tile_pool(name="ypsum", bufs=2, space="PSUM") as ypsum,
        ):
            # Sync queue carries the latency-critical stream in strict
            # priority order: x0, W1 waves, x1, W2, x2.., xo. The queues share
            # ~220 GB/s of HBM read bandwidth, so nothing else may compete
            # early. Scalar queue: biases now, overflow weights later, y-out.
            x_t = [None] * NCH
            x_t[0] = xpool.tile([P, KO, CC], bf16, tag="x", name="x_sb")
            w1_t = {}
            # interleave x0 and wave-0 per ko so the first mm1 group starts
            # as soon as its own (x piece, weight tile) pair has landed
            for ko in range(KO):
                nc.sync.dma_start(x_t[0][:, ko, :sizes[0]],
                                  xr[:, 0, ko, :sizes[0]])
                t = wpool.tile([P, 512], bf16, tag=f"w1_{ko}_0", name="w1_sb")
                nc.sync.dma_start(t[:], w1r[:, ko, 0:512])
                w1_t[(ko, 0)] = t
            for fb in range(1, FB):
                for ko in range(KO):
                    t = wpool.tile([P, 512], bf16, tag=f"w1_{ko}_{fb}")
                    nc.sync.dma_start(t[:], w1r[:, ko, fb * 512:(fb + 1) * 512])
                    w1_t[(ko, fb)] = t

            b1_sb = wpool.tile([P, FT], fp32)
            nc.scalar.dma_start(b1_sb[:], b1[:])
            b2_sb = wpool.tile([P, DT], fp32)
            nc.scalar.dma_start(b2_sb[:], b2[:])
            if KOV:
                b1o_sb = wpool.tile([P, KOV * OFT], fp32)
                nc.scalar.dma_start(b1o_sb[:], b1o[:])
                b2o_sb = wpool.tile([P, KOV * DT], fp32)
                nc.scalar.dma_start(b2o_sb[:], b2o[:])

            w2_t = [None] * FT
            w1o_sb = w2o_sb = None

            def mm1_group(cn, x_sb, ft):
                h_ps = hpsum.tile([P, CC], fp32, tag="hps")
                fb, fc = divmod(ft * P, 512)
                for ko in range(KO):
                    nc.tensor.matmul(
                        h_ps[:, :cn],
                        w1_t[(ko, fb)][:, fc:fc + P],
                        x_sb[:, ko, :cn],
                        start=(ko == 0),
                        stop=(ko == KO - 1),
                    )
                return h_ps

            def gelu(cn, h_ps, h_sb, ft, ft_sb):
                nc.scalar.activation(
                    h_sb[:, ft_sb, :cn], h_ps[:, :cn], AF.Gelu,
                    bias=b1_sb[:, ft:ft + 1],
                )

            def mm2(ci, cn, c0, srcs):
                # srcs: fo -> (sbuf tile, index in tile)
                for dt_ in range(DT):
                    y_ps = ypsum.tile([P, CC], fp32, tag="yps")
                    for fo in range(FT):
                        t, fi = srcs[fo]
                        nc.tensor.matmul(
                            y_ps[:, :cn],
                            w2_t[fo][:, dt_ * P:(dt_ + 1) * P],
                            t[:, fi, :cn],
                            start=(fo == 0),
                            stop=(fo == FT - 1),
                        )
                    y_sb = ypool.tile([P, CC], bf16, tag="y")
                    nc.vector.tensor_scalar_add(
                        y_sb[:, :cn], y_ps[:, :cn], b2_sb[:, dt_:dt_ + 1]
                    )
                    nc.scalar.dma_start(yr[:, dt_, c0:c0 + cn], y_sb[:, :cn])

            # ---- own-expert phase, software-pipelined
            offs = [sum(sizes[:i]) for i in range(NCH)]
            h2_sb = hpool.tile([P, LOOK1, CC], bf16, tag="h2")
            h_prev = None   # (ci, cn, c0, srcs)
            for ci, cn in enumerate(sizes):
                if ci > 0:
                    x_t[ci] = xpool.tile([P, KO, CC], bf16, tag="x", name="x_sb")
                    nc.sync.dma_start(x_t[ci][:, :, :cn], xr[:, ci, :, :cn])
                if ci == 1:
                    # W2 lands behind W1 + x1; needed from mm2(chunk 0) on
                    for fo in range(FT):
                        t = wpool.tile([P, D], bf16, tag=f"w2_{fo}", name="w2_sb")
                        nc.sync.dma_start(t[:], w2r[:, fo, :])
                        w2_t[fo] = t
                if ci == min(2, NCH - 1) and KOV:
                    w1o_sb = wpool.tile([P, KOV, KO, FLOC], bf16, tag="w1o")
                    nc.scalar.dma_start(w1o_sb[:], w1or[:])
                    w2o_sb = wpool.tile([P, KOV, OFT, D], bf16, tag="w2o")
                    nc.scalar.dma_start(w2o_sb[:], w2or[:])
                h_sb = hpool.tile([P, FT, CC], bf16, tag="h")
                srcs = {}
                if h_prev is None:
                    look = 0
                elif ci == 1:
                    look = min(LOOK1, FT)
                else:
                    look = min(LOOK, FT)
                if ci == 0:
                    # all of chunk-0's mm1 runs ko-outer per wave block, so
                    # each arriving (x piece, w1 tile) feeds matmuls at the
                    # DMA drip pace and the PE never idles long enough to
                    # re-throttle
                    for b in range(FT // BLK0):
                        blk = []
                        for ft in range(BLK0):
                            h_ps = hpsum.tile([P, CC], fp32, tag="hps")
                            blk.append(h_ps)
                        for ko in range(KO):
                            for fi_ in range(BLK0):
                                ft = b * BLK0 + fi_
                                nc.tensor.matmul(
                                    blk[fi_][:, :cn],
                                    w1_t[(ko, b)][:, fi_ * P:(fi_ + 1) * P],
                                    x_t[0][:, ko, :cn],
                                    start=(ko == 0),
                                    stop=(ko == KO - 1),
                                )
                        for fi_ in range(BLK0):
                            ft = b * BLK0 + fi_
                            gelu(cn, blk[fi_], h_sb, ft, ft)
                            srcs[ft] = (h_sb, ft)
                    look = FT
                elif ci == 1:
                    # deep lookahead delaying mm2(chunk 0) past the w2
                    # stream: LOOK1 groups with immediate gelu into h2, then
                    # 5 more held in PSUM with gelu deferred past mm2
                    for ft in range(look):
                        h_ps = mm1_group(cn, x_t[ci], ft)
                        gelu(cn, h_ps, h2_sb, ft, ft)
                        srcs[ft] = (h2_sb, ft)
                    held = []
                    for ft in range(look, min(look + 5, FT)):
                        held.append(mm1_group(cn, x_t[ci], ft))
                    mm2(*h_prev)
                    for j, h_ps in enumerate(held):
                        gelu(cn, h_ps, h_sb, look + j, look + j)
                        srcs[look + j] = (h_sb, look + j)
                    look = min(look + 5, FT)
                else:
                    held = []
                    for ft in range(look):
                        held.append(mm1_group(cn, x_t[ci], ft))
                    if h_prev is not None:
                        mm2(*h_prev)
                    for ft in range(look):
                        gelu(cn, held[ft], h_sb, ft, ft)
                        srcs[ft] = (h_sb, ft)
                for ft in range(look, FT):
                    h_ps = mm1_group(cn, x_t[ci], ft)
                    gelu(cn, h_ps, h_sb, ft, ft)
                    srcs[ft] = (h_sb, ft)
                h_prev = (ci, cn, offs[ci], srcs)

            def mm2o(si, o0, cn, oh_sb):
                for dt_ in range(DT):
                    y_ps = ypsum.tile([P, CC], fp32, tag="yps")
                    for fo in range(OFT):
                        nc.tensor.matmul(
                            y_ps[:, :cn],
                            w2o_sb[:, si, fo, dt_ * P:(dt_ + 1) * P],
                            oh_sb[:, fo, :cn],
                            start=(fo == 0),
                            stop=(fo == OFT - 1),
                        )
                    y_sb = ypool.tile([P, CC], bf16, tag="y")
                    nc.vector.tensor_scalar_add(
                        y_sb[:, :cn], y_ps[:, :cn],
                        b2o_sb[:, si * DT + dt_:si * DT + dt_ + 1],
                    )
                    nc.scalar.dma_start(yor[:, dt_, o0:o0 + cn], y_sb[:, :cn])

            # ---- overflow phase (F-sharded partials), pipelined: each
            # of-chunk's mm1 is the lookahead covering the previous mm2
            of_prev = None
            for oc, (si, o0, cn) in enumerate(of_chunks):
                xo_sb = xpool.tile([P, KO, CC], bf16, tag="x")
                nc.sync.dma_start(xo_sb[:, :, :cn], xor_[:, oc, :, :cn])
                oh_sb = hpool.tile([P, LOOK1, CC], bf16, tag="h2")
                for ft in range(OFT):
                    h_ps = hpsum.tile([P, CC], fp32, tag="hps")
                    for ko in range(KO):
                        nc.tensor.matmul(
                            h_ps[:, :cn],
                            w1o_sb[:, si, ko, ft * P:(ft + 1) * P],
                            xo_sb[:, ko, :cn],
                            start=(ko == 0),
                            stop=(ko == KO - 1),
                        )
                    nc.scalar.activation(
                        oh_sb[:, ft, :cn], h_ps[:, :cn], AF.Gelu,
                        bias=b1o_sb[:, si * OFT + ft:si * OFT + ft + 1],
                    )
                if oc == 0:
                    mm2(*h_prev)
                    h_prev = None
                mm2o(si, o0, cn, oh_sb)
            if h_prev is not None:
                mm2(*h_prev)

    return _split_drain_waits(nc)


def _to_bf16(a):
    """Fast float32 -> bfloat16 with round-to-nearest-even via bit ops."""
    a = np.ascontiguousarray(a, dtype=np.float32)
    u = a.view(np.uint32)
    r = ((u + 0x7FFF + ((u >> 16) & 1)) >> 16).astype(np.uint16)
    return r.view(ml_dtypes.bfloat16)


def _pack_cols(mat_bf16, sizes):
    """mat [CT, D] bf16 -> [P, nch, KO, CC] padded chunk regions."""
    out = np.zeros((P, len(sizes), KO, CC), dtype=ml_dtypes.bfloat16)
    c0 = 0
    for i, cn in enumerate(sizes):
        blk = mat_bf16[c0:c0 + cn].T                  # [D, cn]
        out[:, i, :, :cn] = blk.reshape(KO, P, cn).transpose(1, 0, 2)
        c0 += cn
    return np.ascontiguousarray(out.reshape(P, -1))


def kernel(hidden_states, Wg, bg, W1, b1, W2, b2):
    from concourse import bass_utils

    hs = np.ascontiguousarray(hidden_states, dtype=np.float32).reshape(B * S, D)

    # ---- Gate on host (float64): softmax over experts, top-2, renormalize
    logits = hs.astype(np.float64) @ np.asarray(Wg, np.float64).T
    logits += np.asarray(bg, np.float64)
    logits -= logits.max(axis=-1, keepdims=True)
    p = np.exp(logits)
    p /= p.sum(axis=-1, keepdims=True)

    i1 = p.argmax(axis=-1)
    rows = np.arange(B * S)
    p1 = p[rows, i1]
    pm = p.copy()
    pm[rows, i1] = -1.0
    i2 = pm.argmax(axis=-1)
    p2 = p[rows, i2]
    denom = p1 + p2
    g1 = (p1 / denom).astype(np.float32)
    g2 = (p2 / denom).astype(np.float32)

    # ---- Dispatch: token ids + combine weights per expert
    ids, cws = [], []
    for e in range(E):
        m1 = np.nonzero(i1 == e)[0]
        m2 = np.nonzero(i2 == e)[0]
        ids.append(np.concatenate([m1, m2]))
        cws.append(np.concatenate([g1[m1], g2[m2]]))
    cnts = tuple(len(x) for x in ids)

    q = _pick_quota(cnts)
    sizes = _chunk_sizes(q)
    ofs = [(e, cnts[e] - q) for e in range(E) if cnts[e] > q]
    KOV = len(ofs)

    key = (q, cnts)
    if key not in _compiled:
        _compiled[key] = _build_nc(q, cnts)
    nc = _compiled[key]

    W1 = np.asarray(W1, np.float32)
    W2 = np.asarray(W2, np.float32)
    b1 = np.asarray(b1, np.float32)
    b2 = np.asarray(b2, np.float32)

    # overflow columns (shared across cores)
    if KOV:
        of_ids = np.concatenate([ids[e][q:] for e, _ in ofs])
        xo_all = _to_bf16(hs[of_ids])
        of_sizes = []
        for e, cnt in ofs:
            n = -(-cnt // CC)
            base, extra = divmod(cnt, n)
            of_sizes += [base + (1 if i < extra else 0) for i in range(n)]
        xo_host = _pack_cols(xo_all, of_sizes)

    in_maps = []
    for c in range(E):
        cnt = min(cnts[c], q)
        xc = np.zeros((q, D), dtype=ml_dtypes.bfloat16)
        xc[:cnt] = _to_bf16(hs[ids[c][:cnt]])
        m = {
            "x": _pack_cols(xc, sizes),
            "w1": np.ascontiguousarray(
                _to_bf16(W1[c]).reshape(KO, P, F).transpose(1, 0, 2).reshape(P, -1)),
            "w2": np.ascontiguousarray(
                _to_bf16(W2[c]).reshape(FT, P, D).transpose(1, 0, 2).reshape(P, -1)),
            "b1": np.ascontiguousarray(b1[c].reshape(FT, P).T),
            "b2": np.ascontiguousarray(b2[c].reshape(DT, P).T),
        }
        if KOV:
            sl = slice(c * FLOC, (c + 1) * FLOC)
            w1oh = np.empty((P, KOV, KO, FLOC), dtype=ml_dtypes.bfloat16)
            w2oh = np.empty((P, KOV, OFT, D), dtype=ml_dtypes.bfloat16)
            b1oh = np.empty((P, KOV, OFT), dtype=np.float32)
            b2oh = np.empty((P, KOV, DT), dtype=np.float32)
            for si, (e, _) in enumerate(ofs):
                w1oh[:, si] = _to_bf16(W1[e][:, sl]).reshape(
                    KO, P, FLOC).transpose(1, 0, 2)
                w2oh[:, si] = _to_bf16(W2[e][sl, :]).reshape(
                    OFT, P, D).transpose(1, 0, 2)
                b1oh[:, si] = b1[e, sl].reshape(OFT, P).T
                b2oh[:, si] = (b2[e] / E).reshape(DT, P).T
            m.update({
                "xo": xo_host,
                "w1o": np.ascontiguousarray(w1oh.reshape(P, -1)),
                "w2o": np.ascontiguousarray(w2oh.reshape(P, -1)),
                "b1o": np.ascontiguousarray(b1oh.reshape(P, -1)),
                "b2o": np.ascontiguousarray(b2oh.reshape(P, -1)),
            })
        in_maps.append(m)

    kwargs = {}
    if TRACE:
        import os as _os
        kwargs = dict(trace=True, trace_cores=list(range(E)))
        if _os.environ.get("MOE_TRACE_DIR"):
            _os.makedirs(_os.environ["MOE_TRACE_DIR"], exist_ok=True)
            kwargs["tmpdir"] = _os.environ["MOE_TRACE_DIR"]
    res = bass_utils.run_bass_kernel_spmd(nc, in_maps, list(range(E)), **kwargs)
    global LAST_RESULTS
    LAST_RESULTS = res

    # ---- Combine: own-phase outputs, then overflow partial sums
    out = np.zeros((B * S, D), dtype=np.float32)
    for e in range(E):
        cnt = min(cnts[e], q)
        ye = res.results[e]["y"].astype(np.float32)
        out[ids[e][:cnt]] += cws[e][:cnt, None] * ye[:, :cnt].T
    if KOV:
        yo_sum = np.zeros((D, sum(c for _, c in ofs)), dtype=np.float32)
        for c in range(E):
            yo_sum += res.results[c]["yo"].astype(np.float32)
        o0 = 0
        for e, cnt in ofs:
            out[ids[e][q:]] += cws[e][q:, None] * yo_sum[:, o0:o0 + cnt].T
            o0 += cnt
    return out.reshape(B, S, D)
